# revision 18
# baseline (speedup 1.0000x reference)
"""Tensor-parallel GQA attention kernel for 8 Trainium2 NeuronCores.

Problem: x[2,2048,2048] -> Attention(16 q heads, 4 kv heads, rotary,
causal) -> out[2,2048,2048].

Sharding: core c handles batch b=c//4 and kv-group g=c%4 (4 q-heads +
1 kv-head). Each core computes its heads' attention output and a
partial O-projection [DIM, S] (output-dim major, fp16); the host sums
the 4 partials per batch and transposes.

On-core dataflow (feature/dim-major so matmul contractions land on the
partition axis; all matmul operands fp16, fp32 PSUM accumulation):
  Phase A (per 512-token chunk): DMA-transpose x (16x [1024,128]
  transposes per 1024-token group, split scalar+sync queues; DMA
  transpose issue cost is ~1.26us fixed per instruction, so fewer+
  bigger is better and two queues halve the serial issue time).
  QT/KT/VT = W.T @ xT with pre-arranged weights (linear DMAs), RoPE
  fully on DVE, V PE-transposed. psum->sbuf projection copies on the
  scalar (ACT) engine.
  Phase C+D fused: per q-chunk qc and head h:
    scoresT[k,q] = KT_tile.T @ QT into [128,1024] psum pairs
    exp via ACT (scale folded), mask on diag chunks (DVE)
    outT[dv,q] += V_tile.T @ attnT
    O-projection matmuls for q-chunk qc-1 woven between kp iterations
    (PE filler while ACT works through exp) and at head boundaries.
    softmax denominators: grouped ones.T @ attnT matmuls (single
    LDWEIGHTS per head) accumulating into a [1,512] tile that lives
    in the score-psum ring (no extra PSUM bank); reciprocal on DVE,
    partition-broadcast on GPSIMD; output copies alternate DVE/ACT
    and fp16 output DMAs ride the sync queue.
"""
import numpy as np

import concourse.bass as bass
import concourse.tile as tile
import concourse.mybir as mybir
from concourse import bacc
from concourse import bass_utils

F32 = mybir.dt.float32
F16 = mybir.dt.float16

DIM = 2048
S = 2048
B = 2
HL = 4           # q heads per core
FT = DIM // 128  # feature tiles
TT = S // 128    # token tiles (128-token granularity)
SCALE = 1.0 / np.sqrt(128.0)

_CACHE = {}


def _build():
    nc = bacc.Bacc("TRN2", target_bir_lowering=False, debug=False,
                   enable_asserts=True, num_devices=8)

    d_x = nc.dram_tensor("x_c", (S, DIM), F16, kind="ExternalInput").ap()
    d_wq = nc.dram_tensor("wq_c", (128, FT, HL * 128), F16,
                          kind="ExternalInput").ap()
    d_wk = nc.dram_tensor("wk_c", (128, FT, 128), F16,
                          kind="ExternalInput").ap()
    d_wv = nc.dram_tensor("wv_c", (128, FT, 128), F16,
                          kind="ExternalInput").ap()
    d_wo = nc.dram_tensor("wo_c", (128, HL, DIM), F16,
                          kind="ExternalInput").ap()
    d_cj = nc.dram_tensor("cjoin", (128, S), F16, kind="ExternalInput").ap()
    d_sj = nc.dram_tensor("sjoin", (128, S), F16, kind="ExternalInput").ap()
    d_mk = nc.dram_tensor("masks", (128, 4, 512), F16,
                          kind="ExternalInput").ap()
    d_id = nc.dram_tensor("ident", (128, 128), F16, kind="ExternalInput").ap()
    d_ot = nc.dram_tensor("ot", (DIM, S), F16, kind="ExternalOutput").ap()

    Exp = mybir.ActivationFunctionType.Exp

    with tile.TileContext(nc) as tc:
        with tc.tile_pool(name="wts", bufs=1) as wp, \
             tc.tile_pool(name="acts", bufs=1) as ap:
            sb_id = wp.tile([128, 128], F16)
            sb_wq = wp.tile([128, FT, HL * 128], F16)
            sb_wk = wp.tile([128, FT, 128], F16)
            sb_wv = wp.tile([128, FT, 128], F16)
            sb_cj = wp.tile([128, S], F16)
            sb_sj = wp.tile([128, S], F16)
            sb_mk = wp.tile([128, 4, 512], F16)
            sb_wo = wp.tile([128, HL, DIM], F16)
            ones16 = wp.tile([128, 1], F16)
            nc.vector.memset(ones16[:], 1.0)

            sb_QT = ap.tile([128, HL, S], F16)
            sb_KT = ap.tile([128, S], F16)
            sb_V = ap.tile([128, TT, 128], F16)
            sb_oT = ap.tile([128, HL, S], F16)

            # ---- Phase A: x DMA-transpose + Q/K/V projections + RoPE
            with tc.tile_pool(name="xT", bufs=2) as xT_p, \
                 tc.tile_pool(name="vt", bufs=2) as vt_p, \
                 tc.tile_pool(name="rope", bufs=2) as rp, \
                 tc.tile_pool(name="ps_tr", bufs=2, space="PSUM") as ps_tr, \
                 tc.tile_pool(name="ps_pj", bufs=3, space="PSUM") as ps_pj:

                def rope(T, c0):
                    # T: [128, 512] fp16 chunk at token offset c0
                    mc = rp.tile([128, 512], F16, tag="mc")
                    ms = rp.tile([128, 512], F16, tag="ms")
                    cjs = sb_cj[:, c0:c0 + 512]
                    sjs = sb_sj[:, c0:c0 + 512]
                    nc.vector.tensor_mul(mc[:], T, cjs)
                    nc.vector.tensor_mul(ms[0:64, :], T[64:128, :], sjs[64:128, :])
                    nc.vector.tensor_mul(ms[64:128, :], T[0:64, :], sjs[0:64, :])
                    nc.vector.tensor_add(T, mc[:], ms[:])

                def xt_dma(xt, g):
                    # one 1024-token group
                    c0 = g * 1024
                    for fi in range(FT):
                        nc.sync.dma_start(
                            xt[:, fi, :],
                            d_x[c0:c0 + 1024, fi * 128:(fi + 1) * 128],
                            transpose=True)

                # transposes FIRST: a DMA_TRANSPOSE waits for every
                # copy-DMA emitted before it (cross-queue xbar tracking),
                # but copies do not wait for prior transposes.
                xt0 = xT_p.tile([128, FT, 1024], F16, tag="xt")
                xt_dma(xt0, 0)
                xt1 = xT_p.tile([128, FT, 1024], F16, tag="xt")
                xt_dma(xt1, 1)
                xts = [xt0, xt1]
                # weights on the scalar HWDGE queue (fast issue), after
                # the transposes in emission order; wq sliced so each
                # transfer is short (long transfers entangle the shared
                # DMA semaphore rotation and stall the transpose stream)
                for fq in range(4):
                    nc.scalar.dma_start(sb_wq[:, fq * 4:(fq + 1) * 4, :],
                                        d_wq[:, fq * 4:(fq + 1) * 4, :])
                nc.scalar.dma_start(sb_wk[:], d_wk)
                nc.scalar.dma_start(sb_wv[:], d_wv)
                nc.scalar.dma_start(sb_id[:], d_id)
                nc.scalar.dma_start(sb_cj[:], d_cj)
                nc.scalar.dma_start(sb_sj[:], d_sj)
                # force the gpsimd extended-inst library to load NOW (it
                # otherwise loads at the first partition_broadcast in
                # phase C, stalling gpsimd ~6.5us there)
                pbs = wp.tile([1, 16], F32)
                pbd = wp.tile([128, 16], F32)
                nc.vector.memset(pbs[:], 1.0)
                nc.gpsimd.partition_broadcast(pbd[:], pbs[:])

                for sc in range(4):
                    s0 = sc * 512
                    xt = xts[sc // 2]
                    xs = (sc % 2) * 512
                    for h in range(HL):
                        pq = ps_pj.tile([128, 512], F32, tag="pj")
                        for fi in range(FT):
                            nc.tensor.matmul(
                                pq[:], sb_wq[:, fi, h * 128:(h + 1) * 128],
                                xt[:, fi, xs:xs + 512], start=(fi == 0),
                                stop=(fi == FT - 1))
                        nc.scalar.copy(sb_QT[:, h, s0:s0 + 512], pq[:])
                        rope(sb_QT[:, h, s0:s0 + 512], s0)
                    pk = ps_pj.tile([128, 512], F32, tag="pj")
                    for fi in range(FT):
                        nc.tensor.matmul(pk[:], sb_wk[:, fi, :],
                                         xt[:, fi, xs:xs + 512],
                                         start=(fi == 0), stop=(fi == FT - 1))
                    nc.scalar.copy(sb_KT[:, s0:s0 + 512], pk[:])
                    rope(sb_KT[:, s0:s0 + 512], s0)
                    pv = ps_pj.tile([128, 512], F32, tag="pj")
                    for fi in range(FT):
                        nc.tensor.matmul(pv[:], sb_wv[:, fi, :],
                                         xt[:, fi, xs:xs + 512],
                                         start=(fi == 0), stop=(fi == FT - 1))
                    vt = vt_p.tile([128, 512], F16)
                    nc.vector.tensor_copy(vt[:], pv[:])
                    for tl in range(4):
                        ti = sc * 4 + tl
                        ptv = ps_tr.tile([128, 128], F16, tag="tr")
                        nc.tensor.transpose(
                            ptv[:], vt[:, tl * 128:(tl + 1) * 128], sb_id[:])
                        nc.vector.tensor_copy(sb_V[:, ti, :], ptv[:])

            # ---- Phase C+D: attention + interleaved O projection
            with tc.tile_pool(name="attn", bufs=10) as at_p, \
                 tc.tile_pool(name="bcst", bufs=2) as bc_p, \
                 tc.tile_pool(name="rcp", bufs=2) as rc_p, \
                 tc.tile_pool(name="otile", bufs=3) as ot_p, \
                 tc.tile_pool(name="ps_sc", bufs=2, space="PSUM") as ps_sc, \
                 tc.tile_pool(name="ps_po", bufs=2, space="PSUM") as ps_po, \
                 tc.tile_pool(name="ps_ot", bufs=2, space="PSUM") as ps_ot:

                # mask/wo loads here: not needed before ~C start, and
                # emitting them late keeps their transfers off the
                # phase-A transpose window
                nc.scalar.dma_start(sb_mk[:], d_mk)
                nc.scalar.dma_start(sb_wo[:], d_wo)

                # pending O-projection micro-ops for q-chunk qc-1:
                # each item is (q0, oi); expanding to 4 dvi-matmuls + copy+DMA
                pend = []
                state = {"pot": None, "dvi": 0, "cnt": 0}

                def oproj_step():
                    # advance the current O-proj chain by one matmul
                    if not pend:
                        return
                    q0, oi = pend[0]
                    dvi = state["dvi"]
                    if dvi == 0:
                        state["pot"] = ps_ot.tile([128, 512], F32, tag="pot",
                                                  name="pot")
                    pot = state["pot"]
                    nc.tensor.matmul(
                        pot[:], sb_wo[:, dvi, oi * 128:(oi + 1) * 128],
                        sb_oT[:, dvi, q0:q0 + 512],
                        start=(dvi == 0), stop=(dvi == HL - 1))
                    if dvi == HL - 1:
                        otc = ot_p.tile([128, 512], F16, tag="otc")
                        if state["cnt"] % 2 == 0:
                            nc.vector.tensor_copy(otc[:], pot[:])
                        else:
                            nc.scalar.copy(otc[:], pot[:])
                        nc.sync.dma_start(
                            d_ot[oi * 128:(oi + 1) * 128, q0:q0 + 512],
                            otc[:])
                        state["cnt"] += 1
                        state["dvi"] = 0
                        pend.pop(0)
                    else:
                        state["dvi"] = dvi + 1

                # reverse q-chunk order: qc=3 is PE-dense; the ACT-bound
                # small chunks then have O-proj filler from finished chunks
                for qi, qc in enumerate((3, 2, 1, 0)):
                    kmax = (qc + 1) * 4
                    q0 = qc * 512
                    for h in range(HL):
                        po = ps_po.tile([128, 512], F32, tag="po")
                        ats = []
                        for kp in range(kmax // 2):
                            psc = ps_sc.tile([128, 1024], F32, tag="sc")
                            at = at_p.tile([128, 1024], F16, tag="at")
                            for half in range(2):
                                ki = kp * 2 + half
                                nc.tensor.matmul(
                                    psc[:, half * 512:(half + 1) * 512],
                                    sb_KT[:, ki * 128:(ki + 1) * 128],
                                    sb_QT[:, h, q0:q0 + 512],
                                    start=True, stop=True)
                            nc.scalar.activation(at[:], psc[:], Exp,
                                                 scale=SCALE)
                            oproj_step()
                            oproj_step()
                            for half in range(2):
                                ki = kp * 2 + half
                                if ki >= qc * 4:
                                    nc.vector.tensor_mul(
                                        at[:, half * 512:(half + 1) * 512],
                                        at[:, half * 512:(half + 1) * 512],
                                        sb_mk[:, ki - qc * 4, :])
                            for half in range(2):
                                ki = kp * 2 + half
                                nc.tensor.matmul(
                                    po[:], sb_V[:, ki, :],
                                    at[:, half * 512:(half + 1) * 512],
                                    start=(ki == 0), stop=(ki == kmax - 1))
                            ats.append(at)
                        # grouped denominator matmuls (one ones-LDWEIGHTS);
                        # the [1,512] sum tile lives in the score-psum ring
                        psum = ps_sc.tile([1, 512], F32, tag="sc")
                        for kp in range(kmax // 2):
                            for half in range(2):
                                ki = kp * 2 + half
                                nc.tensor.matmul(
                                    psum[:], ones16[:],
                                    ats[kp][:, half * 512:(half + 1) * 512],
                                    start=(ki == 0), stop=(ki == kmax - 1))
                        rc = rc_p.tile([1, 512], F32, tag="rc")
                        nc.vector.reciprocal_approx_fast(rc[:], psum[:])
                        bc = bc_p.tile([128, 512], F32, tag="bc")
                        nc.gpsimd.partition_broadcast(bc[:], rc[:])
                        nc.vector.tensor_mul(
                            sb_oT[:, h, q0:q0 + 512], po[:], bc[:])
                        # head boundary: drain up to 4 whole O-proj chains
                        for _ in range(4 * HL):
                            if not pend and state["dvi"] == 0:
                                break
                            oproj_step()
                    # stage O-projection work for this q-chunk
                    pend.extend((q0, oi) for oi in range(FT))
                    if qi == 3:
                        while pend or state["dvi"] != 0:
                            oproj_step()

    nc.compile()
    return nc


def _prep_shards(x, freqs_cos, freqs_sin, wq, wk, wv, wo):
    perm = np.empty(128, dtype=np.int64)
    perm[0:64] = 2 * np.arange(64)
    perm[64:128] = 2 * np.arange(64) + 1

    cosT = np.ascontiguousarray(freqs_cos.T).astype(np.float32)
    sinT = np.ascontiguousarray(freqs_sin.T).astype(np.float32)
    cjoin = np.concatenate([cosT, cosT], axis=0).astype(np.float16)
    sjoin = np.concatenate([sinT, -sinT], axis=0).astype(np.float16)

    masks = np.zeros((4, 128, 512), dtype=np.float16)
    q_idx = np.arange(512)[None, :]
    k_idx = np.arange(128)[:, None]
    for m in range(4):
        masks[m] = (q_idx >= m * 128 + k_idx).astype(np.float16)
    # pre-arranged [p, m, n] so the DMA is linear
    masks_pm = np.ascontiguousarray(masks.transpose(1, 0, 2))
    ident = np.eye(128, dtype=np.float16)

    in_maps = []
    for c in range(8):
        b, g = c // 4, c % 4
        wq_g = (wq[:, g * 512:(g + 1) * 512].reshape(DIM, 4, 128)[:, :, perm]
                .reshape(DIM, 512))
        # [dim, m] -> [p, ft, m] so the DMA is linear
        wq_g = np.ascontiguousarray(
            wq_g.reshape(FT, 128, 512).transpose(1, 0, 2)).astype(np.float16)
        wk_g = wk[:, g * 128:(g + 1) * 128][:, perm]
        wk_g = np.ascontiguousarray(
            wk_g.reshape(FT, 128, 128).transpose(1, 0, 2)).astype(np.float16)
        wv_g = wv[:, g * 128:(g + 1) * 128]
        wv_g = np.ascontiguousarray(
            wv_g.reshape(FT, 128, 128).transpose(1, 0, 2)).astype(np.float16)
        # wo rows for this group: [512, DIM] -> [p, dv, DIM]
        wo_g = wo[g * 512:(g + 1) * 512, :]
        wo_g = np.ascontiguousarray(
            wo_g.reshape(HL, 128, DIM).transpose(1, 0, 2)).astype(np.float16)
        in_maps.append({
            "x_c": np.ascontiguousarray(x[b]).astype(np.float16),
            "wq_c": wq_g, "wk_c": wk_g, "wv_c": wv_g, "wo_c": wo_g,
            "cjoin": cjoin, "sjoin": sjoin, "masks": masks_pm, "ident": ident,
        })
    return in_maps


def _assemble(results):
    out = np.zeros((B, S, DIM), dtype=np.float32)
    for c in range(8):
        out[c // 4] += results[c]["ot"].T
    return out


def kernel(x, freqs_cos, freqs_sin, wq, wk, wv, wo):
    x = np.asarray(x, dtype=np.float32)
    if "nc" not in _CACHE:
        _CACHE["nc"] = _build()
    nc = _CACHE["nc"]
    in_maps = _prep_shards(x, np.asarray(freqs_cos), np.asarray(freqs_sin),
                           np.asarray(wq), np.asarray(wk), np.asarray(wv),
                           np.asarray(wo))
    res = bass_utils.run_bass_kernel_spmd(nc, in_maps, core_ids=list(range(8)))
    return _assemble(res.results)


# revision 19
# speedup vs baseline: 1.0482x; 1.0482x over previous
"""Tensor-parallel GQA attention kernel for 8 Trainium2 NeuronCores.

Problem: x[2,2048,2048] -> Attention(16 q heads, 4 kv heads, rotary,
causal) -> out[2,2048,2048].

Sharding: core c handles batch b=c//4 and kv-group g=c%4 (4 q-heads +
1 kv-head). Each core computes its heads' attention output and a
partial O-projection [DIM, S] (output-dim major, fp16); the host sums
the 4 partials per batch and transposes.

On-core dataflow (feature/dim-major so matmul contractions land on the
partition axis; all matmul operands fp16, fp32 PSUM accumulation):
  Phase A (per 512-token chunk): DMA-transpose x (16x [1024,128]
  transposes per 1024-token group, split scalar+sync queues; DMA
  transpose issue cost is ~1.26us fixed per instruction, so fewer+
  bigger is better and two queues halve the serial issue time).
  QT/KT/VT = W.T @ xT with pre-arranged weights (linear DMAs), RoPE
  fully on DVE, V PE-transposed. psum->sbuf projection copies on the
  scalar (ACT) engine.
  Phase C+D fused: per q-chunk qc and head h:
    scoresT[k,q] = KT_tile.T @ QT into [128,1024] psum pairs
    exp via ACT (scale folded), mask on diag chunks (DVE)
    outT[dv,q] += V_tile.T @ attnT
    O-projection matmuls for q-chunk qc-1 woven between kp iterations
    (PE filler while ACT works through exp) and at head boundaries.
    softmax denominators: grouped ones.T @ attnT matmuls (single
    LDWEIGHTS per head) accumulating into a [1,512] tile that lives
    in the score-psum ring (no extra PSUM bank); reciprocal on DVE,
    partition-broadcast on GPSIMD; output copies alternate DVE/ACT
    and fp16 output DMAs ride the sync queue.
"""
import numpy as np

import concourse.bass as bass
import concourse.tile as tile
import concourse.mybir as mybir
from concourse import bacc
from concourse import bass_utils

F32 = mybir.dt.float32
F16 = mybir.dt.float16

DIM = 2048
S = 2048
B = 2
HL = 4           # q heads per core
FT = DIM // 128  # feature tiles
TT = S // 128    # token tiles (128-token granularity)
SCALE = 1.0 / np.sqrt(128.0)

_CACHE = {}


def _build():
    nc = bacc.Bacc("TRN2", target_bir_lowering=False, debug=False,
                   enable_asserts=True, num_devices=8)

    d_x = nc.dram_tensor("x_c", (S, DIM), F16, kind="ExternalInput").ap()
    d_wq = nc.dram_tensor("wq_c", (128, FT, HL * 128), F16,
                          kind="ExternalInput").ap()
    d_wk = nc.dram_tensor("wk_c", (128, FT, 128), F16,
                          kind="ExternalInput").ap()
    d_wv = nc.dram_tensor("wv_c", (128, FT, 128), F16,
                          kind="ExternalInput").ap()
    d_wo = nc.dram_tensor("wo_c", (128, HL, DIM), F16,
                          kind="ExternalInput").ap()
    d_cj = nc.dram_tensor("cjoin", (128, S), F16, kind="ExternalInput").ap()
    d_sj = nc.dram_tensor("sjoin", (128, S), F16, kind="ExternalInput").ap()
    d_mk = nc.dram_tensor("masks", (128, 4, 512), F16,
                          kind="ExternalInput").ap()
    d_id = nc.dram_tensor("ident", (128, 128), F16, kind="ExternalInput").ap()
    d_ot = nc.dram_tensor("ot", (DIM, S), F16, kind="ExternalOutput").ap()

    Exp = mybir.ActivationFunctionType.Exp

    with tile.TileContext(nc) as tc:
        with tc.tile_pool(name="wts", bufs=1) as wp, \
             tc.tile_pool(name="acts", bufs=1) as ap:
            sb_id = wp.tile([128, 128], F16)
            sb_wq = wp.tile([128, FT, HL * 128], F16)
            sb_wk = wp.tile([128, FT, 128], F16)
            sb_wv = wp.tile([128, FT, 128], F16)
            sb_cj = wp.tile([128, S], F16)
            sb_sj = wp.tile([128, S], F16)
            sb_mk = wp.tile([128, 4, 512], F16)
            sb_wo = wp.tile([128, HL, DIM], F16)
            ones16 = wp.tile([128, 1], F16)
            nc.vector.memset(ones16[:], 1.0)

            sb_QT = ap.tile([128, HL, S], F16)
            sb_KT = ap.tile([128, S], F16)
            sb_V = ap.tile([128, TT, 128], F16)
            sb_oT = ap.tile([128, HL, S], F16)

            # ---- Phase A: x DMA-transpose + Q/K/V projections + RoPE
            with tc.tile_pool(name="xT", bufs=2) as xT_p, \
                 tc.tile_pool(name="vt", bufs=2) as vt_p, \
                 tc.tile_pool(name="rope", bufs=2) as rp, \
                 tc.tile_pool(name="ps_tr", bufs=2, space="PSUM") as ps_tr, \
                 tc.tile_pool(name="ps_pj", bufs=3, space="PSUM") as ps_pj:

                def rope(T, c0):
                    # T: [128, 512] fp16 chunk at token offset c0
                    mc = rp.tile([128, 512], F16, tag="mc")
                    ms = rp.tile([128, 512], F16, tag="ms")
                    cjs = sb_cj[:, c0:c0 + 512]
                    sjs = sb_sj[:, c0:c0 + 512]
                    nc.vector.tensor_mul(mc[:], T, cjs)
                    nc.vector.tensor_mul(ms[0:64, :], T[64:128, :], sjs[64:128, :])
                    nc.vector.tensor_mul(ms[64:128, :], T[0:64, :], sjs[0:64, :])
                    nc.vector.tensor_add(T, mc[:], ms[:])

                def xt_dma(xt, g):
                    # one 1024-token group
                    c0 = g * 1024
                    for fi in range(FT):
                        nc.sync.dma_start(
                            xt[:, fi, :],
                            d_x[c0:c0 + 1024, fi * 128:(fi + 1) * 128],
                            transpose=True)

                # transposes FIRST: a DMA_TRANSPOSE waits for every
                # copy-DMA emitted before it (cross-queue xbar tracking),
                # but copies do not wait for prior transposes.
                xt0 = xT_p.tile([128, FT, 1024], F16, tag="xt")
                xt_dma(xt0, 0)
                xt1 = xT_p.tile([128, FT, 1024], F16, tag="xt")
                xt_dma(xt1, 1)
                xts = [xt0, xt1]
                # weights on the scalar HWDGE queue (fast issue), after
                # the transposes in emission order
                nc.scalar.dma_start(sb_wq[:], d_wq)
                nc.scalar.dma_start(sb_wk[:], d_wk)
                nc.scalar.dma_start(sb_wv[:], d_wv)
                nc.scalar.dma_start(sb_id[:], d_id)
                nc.scalar.dma_start(sb_cj[:], d_cj)
                nc.scalar.dma_start(sb_sj[:], d_sj)
                nc.scalar.dma_start(sb_mk[:], d_mk)
                # force the gpsimd extended-inst library to load NOW (it
                # otherwise loads at the first partition_broadcast in
                # phase C, stalling gpsimd ~6.5us there)
                pbs = wp.tile([1, 16], F32)
                pbd = wp.tile([128, 16], F32)
                nc.vector.memset(pbs[:], 1.0)
                nc.gpsimd.partition_broadcast(pbd[:], pbs[:])

                for sc in range(4):
                    s0 = sc * 512
                    xt = xts[sc // 2]
                    xs = (sc % 2) * 512
                    for h in range(HL):
                        pq = ps_pj.tile([128, 512], F32, tag="pj")
                        for fi in range(FT):
                            nc.tensor.matmul(
                                pq[:], sb_wq[:, fi, h * 128:(h + 1) * 128],
                                xt[:, fi, xs:xs + 512], start=(fi == 0),
                                stop=(fi == FT - 1))
                        nc.scalar.copy(sb_QT[:, h, s0:s0 + 512], pq[:])
                        rope(sb_QT[:, h, s0:s0 + 512], s0)
                    pk = ps_pj.tile([128, 512], F32, tag="pj")
                    for fi in range(FT):
                        nc.tensor.matmul(pk[:], sb_wk[:, fi, :],
                                         xt[:, fi, xs:xs + 512],
                                         start=(fi == 0), stop=(fi == FT - 1))
                    nc.scalar.copy(sb_KT[:, s0:s0 + 512], pk[:])
                    rope(sb_KT[:, s0:s0 + 512], s0)
                    pv = ps_pj.tile([128, 512], F32, tag="pj")
                    for fi in range(FT):
                        nc.tensor.matmul(pv[:], sb_wv[:, fi, :],
                                         xt[:, fi, xs:xs + 512],
                                         start=(fi == 0), stop=(fi == FT - 1))
                    vt = vt_p.tile([128, 512], F16)
                    nc.vector.tensor_copy(vt[:], pv[:])
                    for tl in range(4):
                        ti = sc * 4 + tl
                        ptv = ps_tr.tile([128, 128], F16, tag="tr")
                        nc.tensor.transpose(
                            ptv[:], vt[:, tl * 128:(tl + 1) * 128], sb_id[:])
                        nc.vector.tensor_copy(sb_V[:, ti, :], ptv[:])
                # wo last (first needed ~10us into C)
                nc.scalar.dma_start(sb_wo[:], d_wo)

            # ---- Phase C+D: attention + interleaved O projection
            with tc.tile_pool(name="attn", bufs=10) as at_p, \
                 tc.tile_pool(name="bcst", bufs=2) as bc_p, \
                 tc.tile_pool(name="rcp", bufs=2) as rc_p, \
                 tc.tile_pool(name="otile", bufs=3) as ot_p, \
                 tc.tile_pool(name="ps_sc", bufs=2, space="PSUM") as ps_sc, \
                 tc.tile_pool(name="ps_po", bufs=2, space="PSUM") as ps_po, \
                 tc.tile_pool(name="ps_ot", bufs=2, space="PSUM") as ps_ot:

                # pending O-projection micro-ops for q-chunk qc-1:
                # each item is (q0, oi); expanding to 4 dvi-matmuls + copy+DMA
                pend = []
                state = {"pot": None, "dvi": 0, "cnt": 0}

                def oproj_step():
                    # advance the current O-proj chain by one matmul
                    if not pend:
                        return
                    q0, oi = pend[0]
                    dvi = state["dvi"]
                    if dvi == 0:
                        state["pot"] = ps_ot.tile([128, 512], F32, tag="pot",
                                                  name="pot")
                    pot = state["pot"]
                    nc.tensor.matmul(
                        pot[:], sb_wo[:, dvi, oi * 128:(oi + 1) * 128],
                        sb_oT[:, dvi, q0:q0 + 512],
                        start=(dvi == 0), stop=(dvi == HL - 1))
                    if dvi == HL - 1:
                        otc = ot_p.tile([128, 512], F16, tag="otc")
                        if state["cnt"] % 2 == 0:
                            nc.vector.tensor_copy(otc[:], pot[:])
                        else:
                            nc.scalar.copy(otc[:], pot[:])
                        nc.sync.dma_start(
                            d_ot[oi * 128:(oi + 1) * 128, q0:q0 + 512],
                            otc[:])
                        state["cnt"] += 1
                        state["dvi"] = 0
                        pend.pop(0)
                    else:
                        state["dvi"] = dvi + 1

                # reverse q-chunk order: qc=3 is PE-dense; the ACT-bound
                # small chunks then have O-proj filler from finished chunks
                for qi, qc in enumerate((3, 2, 1, 0)):
                    kmax = (qc + 1) * 4
                    q0 = qc * 512
                    for h in range(HL):
                        po = ps_po.tile([128, 512], F32, tag="po")
                        ats = []
                        for kp in range(kmax // 2):
                            psc = ps_sc.tile([128, 1024], F32, tag="sc")
                            at = at_p.tile([128, 1024], F16, tag="at")
                            for half in range(2):
                                ki = kp * 2 + half
                                nc.tensor.matmul(
                                    psc[:, half * 512:(half + 1) * 512],
                                    sb_KT[:, ki * 128:(ki + 1) * 128],
                                    sb_QT[:, h, q0:q0 + 512],
                                    start=True, stop=True)
                            nc.scalar.activation(at[:], psc[:], Exp,
                                                 scale=SCALE)
                            oproj_step()
                            oproj_step()
                            for half in range(2):
                                ki = kp * 2 + half
                                if ki >= qc * 4:
                                    nc.vector.tensor_mul(
                                        at[:, half * 512:(half + 1) * 512],
                                        at[:, half * 512:(half + 1) * 512],
                                        sb_mk[:, ki - qc * 4, :])
                            for half in range(2):
                                ki = kp * 2 + half
                                nc.tensor.matmul(
                                    po[:], sb_V[:, ki, :],
                                    at[:, half * 512:(half + 1) * 512],
                                    start=(ki == 0), stop=(ki == kmax - 1))
                            ats.append(at)
                        # grouped denominator matmuls (one ones-LDWEIGHTS);
                        # the [1,512] sum tile lives in the score-psum ring
                        psum = ps_sc.tile([1, 512], F32, tag="sc")
                        for kp in range(kmax // 2):
                            for half in range(2):
                                ki = kp * 2 + half
                                nc.tensor.matmul(
                                    psum[:], ones16[:],
                                    ats[kp][:, half * 512:(half + 1) * 512],
                                    start=(ki == 0), stop=(ki == kmax - 1))
                        rc = rc_p.tile([1, 512], F32, tag="rc")
                        nc.vector.reciprocal_approx_fast(rc[:], psum[:])
                        bc = bc_p.tile([128, 512], F32, tag="bc")
                        nc.gpsimd.partition_broadcast(bc[:], rc[:])
                        nc.vector.tensor_mul(
                            sb_oT[:, h, q0:q0 + 512], po[:], bc[:])
                        # head boundary: drain up to 4 whole O-proj chains
                        for _ in range(4 * HL):
                            if not pend and state["dvi"] == 0:
                                break
                            oproj_step()
                    # stage O-projection work for this q-chunk
                    pend.extend((q0, oi) for oi in range(FT))
                    if qi == 3:
                        while pend or state["dvi"] != 0:
                            oproj_step()

    nc.compile()
    return nc


def _prep_shards(x, freqs_cos, freqs_sin, wq, wk, wv, wo):
    perm = np.empty(128, dtype=np.int64)
    perm[0:64] = 2 * np.arange(64)
    perm[64:128] = 2 * np.arange(64) + 1

    cosT = np.ascontiguousarray(freqs_cos.T).astype(np.float32)
    sinT = np.ascontiguousarray(freqs_sin.T).astype(np.float32)
    cjoin = np.concatenate([cosT, cosT], axis=0).astype(np.float16)
    sjoin = np.concatenate([sinT, -sinT], axis=0).astype(np.float16)

    masks = np.zeros((4, 128, 512), dtype=np.float16)
    q_idx = np.arange(512)[None, :]
    k_idx = np.arange(128)[:, None]
    for m in range(4):
        masks[m] = (q_idx >= m * 128 + k_idx).astype(np.float16)
    # pre-arranged [p, m, n] so the DMA is linear
    masks_pm = np.ascontiguousarray(masks.transpose(1, 0, 2))
    ident = np.eye(128, dtype=np.float16)

    in_maps = []
    for c in range(8):
        b, g = c // 4, c % 4
        wq_g = (wq[:, g * 512:(g + 1) * 512].reshape(DIM, 4, 128)[:, :, perm]
                .reshape(DIM, 512))
        # [dim, m] -> [p, ft, m] so the DMA is linear
        wq_g = np.ascontiguousarray(
            wq_g.reshape(FT, 128, 512).transpose(1, 0, 2)).astype(np.float16)
        wk_g = wk[:, g * 128:(g + 1) * 128][:, perm]
        wk_g = np.ascontiguousarray(
            wk_g.reshape(FT, 128, 128).transpose(1, 0, 2)).astype(np.float16)
        wv_g = wv[:, g * 128:(g + 1) * 128]
        wv_g = np.ascontiguousarray(
            wv_g.reshape(FT, 128, 128).transpose(1, 0, 2)).astype(np.float16)
        # wo rows for this group: [512, DIM] -> [p, dv, DIM]
        wo_g = wo[g * 512:(g + 1) * 512, :]
        wo_g = np.ascontiguousarray(
            wo_g.reshape(HL, 128, DIM).transpose(1, 0, 2)).astype(np.float16)
        in_maps.append({
            "x_c": np.ascontiguousarray(x[b]).astype(np.float16),
            "wq_c": wq_g, "wk_c": wk_g, "wv_c": wv_g, "wo_c": wo_g,
            "cjoin": cjoin, "sjoin": sjoin, "masks": masks_pm, "ident": ident,
        })
    return in_maps


def _assemble(results):
    out = np.zeros((B, S, DIM), dtype=np.float32)
    for c in range(8):
        out[c // 4] += results[c]["ot"].T
    return out


def kernel(x, freqs_cos, freqs_sin, wq, wk, wv, wo):
    x = np.asarray(x, dtype=np.float32)
    if "nc" not in _CACHE:
        _CACHE["nc"] = _build()
    nc = _CACHE["nc"]
    in_maps = _prep_shards(x, np.asarray(freqs_cos), np.asarray(freqs_sin),
                           np.asarray(wq), np.asarray(wk), np.asarray(wv),
                           np.asarray(wo))
    res = bass_utils.run_bass_kernel_spmd(nc, in_maps, core_ids=list(range(8)))
    return _assemble(res.results)
